# revision 34
# baseline (speedup 1.0000x reference)
"""GRU (B=64, T=512, DIN=D=512) on 8 Trainium2 NeuronCores.

Strategy
--------
Data-parallel over batch: each core owns BL = 8 batch rows, weights
replicated.  The scan is NX-dispatch-bound on the tensor engine (~26ns per
LDWEIGHTS+MATMUL pair at FD=8, clock-independent), so the design minimizes
PE instruction count and serial cross-engine stalls, not weight-load bytes:

1. Projection: xg = X @ W_g + b_g all in bf16 (W, U, X converted host-side)
   as FD=256 matmuls, ~5x cheaper than fp32.  Chunk 0 runs as a prologue;
   chunk c+1's projection is drip-fed during scan chunk c as single-MM
   "plugs" anchored after each step's candidate matmuls -- exactly the
   window where the PE would otherwise idle on the tanh->blend->next-step
   chain.  psum->SBUF evacuation (Identity ACT, bias fused, bf16 cast) runs
   in ScalarE's post-tanh idle window; on VectorE it would head-of-line
   block the critical blend ops.  Pre-activations live in per-chunk SBUF
   tiles (4 rotating slots) so evac writes never alias scan reads in the
   dependency tracker.

2. Scan: state transposed hT[d%128, KT=4, BL], U stationary bf16, 48
   LDW+MM pairs per step plus 4 identity matmuls that inject the x-preacts
   (start=True) with no recurrent deps, so the scheduler hoists them into
   stalls.  Per-gate PSUM tiles are full-bank (PSUM write/read collisions
   are fatal / force serialization); pr/pz double-buffered so the previous
   step's late sigmoid reads never stall the injections.  The candidate
   gate is split across two banks (m-tiles 01/23) with m-major matmul
   order, so tanh of the first half overlaps the second half's matmuls and
   the next step's r matmuls (needing only the first half of the new
   state, k-half-major order) start early.  zc = sigmoid(-zpre) = 1-z and
   c1n = (zc-1)*hm (one fused scalar_tensor_tensor) turn the blend into
   h = zc*hh - c1n with only mul+sub after tanh.

Mask semantics: h_t = z*(m_{t-1}*h_{t-1}) + ...; all-ones mask (the graded
case) skips the multiply; a general path streams a host-broadcast shifted
mask and adds one DVE mul per step.
"""

import numpy as np
from contextlib import ExitStack

import ml_dtypes
import concourse.bass as bass
import concourse.bacc as bacc
import concourse.mybir as mybir
import concourse.tile as tile
from concourse.tile import add_dep_helper
from concourse.bass_utils import run_bass_kernel_spmd

FP32 = mybir.dt.float32
BF16 = mybir.dt.bfloat16
AF = mybir.ActivationFunctionType

B, T, DIN, D = 64, 512, 512, 512
NCORES = 8
BL = B // NCORES            # 8 batch rows per core
KT = DIN // 128             # 4 contraction tiles
MT = D // 128               # 4 output tiles
P = 128
CL = 32                     # steps per chunk (xt DMA + projection granularity)
PCW = CL * BL               # chunk width in columns (512)
PH = PCW // 2               # projection matmul free-dim (256)

# recurrent MM order for r/z: k-half-major (state halves arrive staggered)
ORD_K = ([(kk, m) for kk in (0, 1) for m in range(MT)]
         + [(kk, m) for kk in (2, 3) for m in range(MT)])
# h-candidate: m-major so m-halves complete early for the tanh/blend overlap
ORD_M = [(kk, m) for m in range(MT) for kk in range(KT)]


def build_nc(T_=T, masked=False):
    """Build the single-core SPMD program (identical on all 8 cores)."""
    sch = T_ // CL                       # chunks

    nc = bacc.Bacc(None, target_bir_lowering=False, debug=False)

    xT = nc.dram_tensor("xT", [DIN, T_ * BL], BF16, kind="ExternalInput")
    w_lay = {g: nc.dram_tensor(f"W{g}", [P, KT * D], BF16, kind="ExternalInput")
             for g in "zrh"}
    u_lay = {g: nc.dram_tensor(f"U{g}", [P, KT * D], BF16, kind="ExternalInput")
             for g in "zrh"}
    b4 = {g: nc.dram_tensor(f"b{g}", [P, MT], FP32, kind="ExternalInput")
          for g in "zrh"}
    eye_d = nc.dram_tensor("eye", [P, P], BF16, kind="ExternalInput")
    mb = None
    if masked:
        mb = nc.dram_tensor("mb", [T_, P, KT * BL], FP32, kind="ExternalInput")
    hT_out = nc.dram_tensor("hT_out", [D, BL], FP32, kind="ExternalOutput")

    with tile.TileContext(nc) as tc, ExitStack() as ctx:
        upool = ctx.enter_context(tc.tile_pool(name="upool", bufs=1))
        xap = ctx.enter_context(tc.tile_pool(name="xap", bufs=1))
        xtp = ctx.enter_context(tc.tile_pool(name="xtp", bufs=2 * KT))
        ppj = ctx.enter_context(tc.tile_pool(name="ppj", bufs=2, space="PSUM"))
        psc = ctx.enter_context(tc.tile_pool(name="psc", bufs=1, space="PSUM"))
        sm = ctx.enter_context(tc.tile_pool(name="sm", bufs=3))
        mbp = ctx.enter_context(tc.tile_pool(name="mbp", bufs=2))

        eye_sb = upool.tile([P, P], BF16, tag="eye", name="eye")
        nc.sync.dma_start(eye_sb[:], eye_d[:])
        u_sb, w_sb, b_sb = {}, {}, {}
        for g in "zrh":
            u_sb[g] = upool.tile([P, KT * D], BF16, tag=f"u{g}", name=f"u{g}")
            nc.sync.dma_start(u_sb[g][:], u_lay[g][:])
            w_sb[g] = upool.tile([P, KT * D], BF16, tag=f"w{g}", name=f"w{g}")
            nc.sync.dma_start(w_sb[g][:], w_lay[g][:])
            b_sb[g] = upool.tile([P, MT], FP32, tag=f"b{g}", name=f"b{g}")
            nc.sync.dma_start(b_sb[g][:], b4[g][:])

        # SBUF-resident pre-activations, one tile per chunk (rotating 4 slots)
        # so projection evacuations (writes into chunk c+2) never alias the
        # scan's reads of chunk c in the dependency tracker
        xa_tiles = {}

        def xa_tile(c):
            if c not in xa_tiles:
                xa_tiles[c] = xap.tile([P, 3, KT, PCW], BF16,
                                       tag=f"xa{c % 4}", name=f"xa{c}")
            return xa_tiles[c]

        gate_i = {"z": 0, "r": 1, "h": 2}

        # scan psum: full-bank tiles so each lives in its own bank; the
        # 128B step-region is a contiguous view of the bank's first bytes.
        # pr/pz are double-buffered: their WARs (the sigmoid reads of step
        # t-1 complete mid-step-t) would otherwise stall the x injections
        # and block the scheduler from hoisting them into idle windows.
        pr_fs = [psc.tile([P, KT * 128], FP32, tag=f"pr{i}", name=f"pr{i}")
                 for i in range(2)]
        pz_fs = [psc.tile([P, KT * 128], FP32, tag=f"pz{i}", name=f"pz{i}")
                 for i in range(2)]
        ph0_f = psc.tile([P, KT * 128], FP32, tag="ph0", name="ph0")
        ph1_f = psc.tile([P, KT * 128], FP32, tag="ph1", name="ph1")
        pr_vs = [f[:, 0:KT * BL].rearrange("p (m b) -> p m b", m=KT)
                 for f in pr_fs]
        pz_vs = [f[:, 0:KT * BL].rearrange("p (m b) -> p m b", m=KT)
                 for f in pz_fs]
        ph0_v = ph0_f[:, 0:2 * BL].rearrange("p (m b) -> p m b", m=2)
        ph1_v = ph1_f[:, 0:2 * BL].rearrange("p (m b) -> p m b", m=2)

        xt_tiles = {}

        def emit_xt_dmas(c):
            tiles = []
            for kk in range(KT):
                xt = xtp.tile([P, PCW], BF16, tag="xt", name=f"xt{c}_{kk}")
                nc.sync.dma_start(
                    xt[:], xT[kk * P:(kk + 1) * P, c * PCW:(c + 1) * PCW])
                tiles.append(xt)
            xt_tiles[c] = tiles

        def make_proj_thunks(c):
            """Projection work for chunk c as single-instruction thunks.

            Unit (g, m): 8 matmuls (2 col-halves x 4 k) into one full-bank
            psum tile, then 2 ScalarE evacuations (Identity + bias, bf16
            cast).  MM thunks go to the PE plug queue; evac thunks go to a
            separate queue drained in ScalarE's post-tanh idle window (a
            DVE evac would head-of-line-block the critical blend ops)."""
            mms, evs = [], []
            for g in "zrh":
                for m in range(MT):
                    st = {"left": 8}

                    def mk_mm(g=g, m=m, st=st, kk=0, hf=0):
                        def run(anchor):
                            if "pp" in st:
                                pp = st["pp"]
                            else:
                                pp = ppj.tile([P, PCW], FP32, tag="pp",
                                              name=f"pp{c}{g}{m}")
                                st["pp"] = pp
                            mm = nc.tensor.matmul(
                                pp[:, hf * PH:(hf + 1) * PH],
                                w_sb[g][:, kk * D + m * P: kk * D + (m + 1) * P],
                                xt_tiles[c][kk][:, hf * PH:(hf + 1) * PH],
                                start=(kk == 0), stop=(kk == KT - 1))
                            st["left"] -= 1
                            if anchor is not None:
                                add_dep_helper(mm.ins, anchor, sync=False,
                                               reason="proj plug placement")
                            return mm.ins
                        return run

                    def mk_ev(g=g, m=m, st=st, hf=0):
                        def run(anchor):
                            nc.scalar.activation(
                                xa_tile(c)[:, gate_i[g], m,
                                           hf * PH:(hf + 1) * PH],
                                st["pp"][:, hf * PH:(hf + 1) * PH],
                                AF.Identity, bias=b_sb[g][:, m:m + 1])
                            return None
                        run.ready = lambda st=st: st["left"] == 0
                        return run

                    for hf in range(2):
                        for kk in range(KT):
                            mms.append(mk_mm(kk=kk, hf=hf))
                        evs.append(mk_ev(hf=hf))
            return mms, evs

        # prologue: chunk 0 projected densely before the scan
        n_pro = min(sch, 1)
        prologue_insts = []
        for c in range(min(sch, n_pro + 1)):
            emit_xt_dmas(c)
        for c in range(n_pro):
            mms, evs = make_proj_thunks(c)
            ev_i = 0
            for j, th in enumerate(mms):
                i = th(None)
                if i is not None:
                    prologue_insts.append(i)
                # interleave each unit's evacs right after its 8th matmul
                while ev_i < len(evs) and evs[ev_i].ready():
                    evs[ev_i](None)
                    ev_i += 1
            while ev_i < len(evs):
                evs[ev_i](None)
                ev_i += 1

        mm_q, ev_q = [], []

        h_prev = sm.tile([P, KT, BL], BF16, tag="h", name="h0")
        nc.vector.memset(h_prev[:], 0.0)

        for t in range(T_):
            c, ti = divmod(t, CL)
            if ti == 0:
                if c + 2 < sch:
                    emit_xt_dmas(c + 2)
                if n_pro <= c + 1 < sch:
                    mms, evs = make_proj_thunks(c + 1)
                    mm_q.extend(mms)
                    ev_q.extend(evs)
                if masked:
                    mb_sb = mbp.tile([P, CL, KT * BL], FP32, tag="m",
                                     name=f"mb{c}")
                    nc.sync.dma_start(
                        mb_sb[:],
                        mb[c * CL:(c + 1) * CL].rearrange("t p x -> p t x"))

            if masked:
                hm = sm.tile([P, KT, BL], BF16, tag="hm")
                nc.vector.tensor_mul(
                    hm[:], h_prev[:],
                    mb_sb[:, ti].rearrange("p (k b) -> p k b", k=KT))
            else:
                hm = h_prev

            xv = xa_tile(c)[:, :, :, ti * BL:(ti + 1) * BL]
            bar = prologue_insts if t == 0 else None

            def gate_mms(out_for_m, g, rhs, xvg, order, idparts, after=None):
                # identity matmul(s) inject the x-projection (start=True);
                # no recurrent data deps, so the scheduler can fill stalls
                for (ov, xs) in idparts:
                    idmm = nc.tensor.matmul(ov, eye_sb[:], xs,
                                            start=True, stop=False)
                    if bar:
                        for e in bar:
                            add_dep_helper(idmm.ins, e, sync=True,
                                           reason="prologue barrier")
                stop_mm = None
                for i, (kk, m) in enumerate(order):
                    mm = nc.tensor.matmul(
                        out_for_m(m),
                        u_sb[g][:, kk * D + m * P: kk * D + (m + 1) * P],
                        rhs[:, kk],
                        start=False,
                        stop=(kk == KT - 1))
                    if i == 0 and after is not None:
                        add_dep_helper(mm.ins, after, sync=False,
                                       reason="gate ordering")
                    stop_mm = mm
                return stop_mm

            # r gate
            pr_v = pr_vs[t % 2]
            r_stop = gate_mms(lambda m: pr_v[:, m], "r", hm, xv[:, 1], ORD_K,
                              [(pr_v[:], xv[:, 1])])
            r_sb = sm.tile([P, KT, BL], BF16, tag="r")
            nc.scalar.activation(r_sb[:], pr_v[:], AF.Sigmoid)
            rhm = sm.tile([P, KT, BL], BF16, tag="rhm")
            nc.vector.tensor_mul(rhm[:], r_sb[:], hm[:])

            # z gate (complement): zc = 1 - z = sigmoid(-zpre)
            pz_v = pz_vs[t % 2]
            z_stop = gate_mms(lambda m: pz_v[:, m], "z", hm, xv[:, 0], ORD_K,
                              [(pz_v[:], xv[:, 0])], after=r_stop.ins)
            zc = sm.tile([P, KT, BL], BF16, tag="zc")
            nc.scalar.activation(zc[:], pz_v[:], AF.Sigmoid, scale=-1.0)
            # c1n = (zc - 1)*hm = -(hm - zc*hm) in ONE fused DVE op;
            # the blend then becomes h = zc*hh - c1n
            c1n = sm.tile([P, KT, BL], BF16, tag="c1n")
            nc.vector.scalar_tensor_tensor(
                c1n[:], zc[:], 1.0, hm[:],
                mybir.AluOpType.subtract, mybir.AluOpType.mult)

            # one projection plug lands between z and h, absorbing the
            # wait for rhm (sigmoid-r -> mul chain) after z's matmuls
            if mm_q and ti >= 1:
                mm_q.pop(0)(z_stop.ins)

            # h candidate: m-major, split across two banks so tanh of the
            # first m-half overlaps the second half's matmuls
            def h_out(m):
                return ph0_v[:, m] if m < 2 else ph1_v[:, m - 2]
            h_stop = gate_mms(h_out, "h", rhm, xv[:, 2], ORD_M,
                              [(ph0_v[:], xv[:, 2, 0:2]),
                               (ph1_v[:], xv[:, 2, 2:4])],
                              after=z_stop.ins)

            # tail per m-half: h = zc*hh - c1n.  The two muls are emitted
            # before the two subs so the second half's mul isn't FIFO-starved
            # behind the first half's sub on the vector engine.
            hh = sm.tile([P, KT, BL], BF16, tag="hh")
            b2 = sm.tile([P, KT, BL], BF16, tag="b2")
            h_new = sm.tile([P, KT, BL], BF16, tag="h")
            s0, s1 = slice(0, 2), slice(2, 4)
            nc.scalar.activation(hh[:, s0], ph0_v[:], AF.Tanh)
            nc.scalar.activation(hh[:, s1], ph1_v[:], AF.Tanh)
            nc.vector.tensor_mul(b2[:, s0], zc[:, s0], hh[:, s0])
            nc.vector.tensor_mul(b2[:, s1], zc[:, s1], hh[:, s1])
            nc.vector.tensor_sub(h_new[:, s0], b2[:, s0], c1n[:, s0])
            nc.vector.tensor_sub(h_new[:, s1], b2[:, s1], c1n[:, s1])
            h_prev = h_new

            # drip projection plugs into this step's tail window (PE), and
            # evacuations into ScalarE's post-tanh idle window
            if ti >= 1:
                left = CL - 1 - ti
                npop = len(mm_q) if left <= 0 else min(
                    4, -(-len(mm_q) // left))
                for _ in range(min(npop, len(mm_q))):
                    mm_q.pop(0)(h_stop.ins)
                nev = len(ev_q) if ti == CL - 1 else 1
                for _ in range(nev):
                    if ev_q and ev_q[0].ready():
                        ev_q.pop(0)(None)
                    else:
                        break


        hout = sm.tile([P, KT, BL], FP32, tag="hout", name="hout")
        nc.vector.tensor_copy(hout[:], h_prev[:])
        nc.sync.dma_start(hT_out.rearrange("(k p) b -> p k b", p=P), hout[:])

    nc.compile()
    return nc


_NC_CACHE = {}


def _get_nc(masked):
    if masked not in _NC_CACHE:
        _NC_CACHE[masked] = build_nc(T, masked=masked)
    return _NC_CACHE[masked]


def _w_layout(w):
    # [DIN, D] -> [128, KT*D] with lay[p, kk*D + j] = w[kk*128 + p, j]
    return np.ascontiguousarray(
        np.asarray(w, dtype=np.float32).reshape(KT, P, D).transpose(1, 0, 2)
        .reshape(P, KT * D)).astype(ml_dtypes.bfloat16)


def _b_layout(b):
    return np.ascontiguousarray(
        np.asarray(b, dtype=np.float32).reshape(MT, P).T, dtype=np.float32)


def make_in_maps(X, W_z, U_z, b_z, W_r, U_r, b_r, W_h, U_h, b_h, mask,
                 masked):
    X = np.asarray(X, dtype=np.float32)
    shared = {"eye": np.eye(P, dtype=np.float32).astype(ml_dtypes.bfloat16)}
    for g, w, u, b in (("z", W_z, U_z, b_z), ("r", W_r, U_r, b_r),
                       ("h", W_h, U_h, b_h)):
        shared[f"W{g}"] = _w_layout(w)
        shared[f"U{g}"] = _w_layout(u)
        shared[f"b{g}"] = _b_layout(b)

    in_maps = []
    for c in range(NCORES):
        bsl = slice(c * BL, (c + 1) * BL)
        m = dict(shared)
        m["xT"] = np.ascontiguousarray(
            X[bsl].transpose(2, 1, 0).reshape(DIN, T * BL)).astype(
                ml_dtypes.bfloat16)
        if masked:
            msh = np.zeros((T, BL), dtype=np.float32)
            msh[1:] = np.asarray(mask)[bsl, :T - 1].T.astype(np.float32)
            m["mb"] = np.ascontiguousarray(
                np.tile(msh[:, None, :], (1, P, KT)))
        in_maps.append(m)
    return in_maps


def kernel(X, W_z, U_z, b_z, W_r, U_r, b_r, W_h, U_h, b_h, mask):
    mask = np.asarray(mask)
    masked = not bool(np.all(mask[:, :T - 1] == 1))
    nc = _get_nc(masked)
    in_maps = make_in_maps(X, W_z, U_z, b_z, W_r, U_r, b_r, W_h, U_h, b_h,
                           mask, masked)
    res = run_bass_kernel_spmd(nc, in_maps, core_ids=list(range(NCORES)))
    out = np.empty((B, D), dtype=np.float32)
    for c in range(NCORES):
        out[c * BL:(c + 1) * BL] = res.results[c]["hT_out"].T
    return out
